# revision 12
# baseline (speedup 1.0000x reference)
"""Trainium2 Bass kernel for nn_BindingSiteGNN (GAT+GCN+SAGE GNN).

Strategy (8 NeuronCores, SPMD):
  Launch A (row-parallel over nodes): input projection h = relu(bn(x@w_in)),
    plus hg = h@w_gat, hsr = h@w_sage_r and attention logits a = h@[v_src|v_dst],
    in feature-major layout. BatchNorms are folded into weights host-side.
  Host reshard: build a per-node gather table [h | hg | a_src] (bf16) plus
    per-edge mask/index data. Edges are grouped by destination into blocks of
    64 dst slots; dsts are bin-packed so every block holds <= 1024 edges and
    every core gets exactly 40 blocks (2560 dst slots).
  Launch B (edge-parallel by dst): per 1024-edge chunk, dma_gather pulls the
    table rows for the chunk's sources; PE aggregates per dst via mask
    matmuls accumulated in PSUM (SAGE/GCN: 0/1 and dinv-weighted host masks;
    GAT: mask0 * exp(attention) built on-chip); a per-dst epilogue applies
    GAT softmax denominators / GCN symmetric norm / SAGE mean, then the
    dense tail (residual MLP + classifier) runs feature-major.
  Host scatters per-core logits back to global node order.
"""
import sys

sys.path.insert(0, '/opt/trn_rl_repo')

from contextlib import ExitStack

import numpy as np
import ml_dtypes

import concourse.bacc as bacc
import concourse.tile as tile
import concourse.masks as cmasks
from concourse import mybir
from concourse.bass_utils import run_bass_kernel_spmd

BF = ml_dtypes.bfloat16
F32 = mybir.dt.float32
BF16 = mybir.dt.bfloat16
I16 = mybir.dt.int16
AF = mybir.ActivationFunctionType
ALU = mybir.AluOpType

N, E = 20000, 320000
D_IN, D_H = 1280, 512
HEADS, CH = 4, 128
BN_EPS = 1e-5
LRELU = 0.2
NCORES = 8

BLK_DST = 64            # dst slots per block
TB = 8                  # tiles (of 128 edges) per block
CHUNK_E = 128 * TB      # 1024 edges per chunk == one block
NBLK = 40               # blocks per core
NPAIR = NBLK // 2
T = NBLK * TB           # 320 tiles per core
NSLOT = NBLK * BLK_DST  # 2560 dst slots per core
NODEPAD = 2560          # padded node rows per core in launch A
TBL_W = 1152            # table row width (bf16 slots): h 512 | hg 512 | extras
EXTRA = 1024            # extras offset (slots)
PAD_ADST = -60.0        # pad edges: drives exp() to ~0

_CACHE = {}


# ======================================================================
# host-side preprocessing
# ======================================================================

def fold_params(params):
    s = np.float32(1.0 / np.sqrt(np.float32(1.0 + BN_EPS)))
    p = {k: np.asarray(v, np.float32) for k, v in params.items() if k != 'res'}
    out = {}
    g = p['bn_in_g'] * s
    out['w_in'] = p['w_in'] * g[None, :]
    out['b_in'] = p['b_in'] * g + p['bn_in_b']
    out['w_gat'] = p['w_gat']
    out['b_gat'] = p['b_gat']
    wg3 = p['w_gat'].reshape(D_H, HEADS, CH)
    out['v_src'] = np.einsum('dhc,hc->dh', wg3, p['att_src']).astype(np.float32)
    out['v_dst'] = np.einsum('dhc,hc->dh', wg3, p['att_dst']).astype(np.float32)
    for k in ('w_gcn', 'b_gcn', 'w_sage_l', 'w_sage_r', 'b_sage', 'wc1', 'bc1'):
        out[k] = p[k]
    out['res'] = []
    for r in params['res']:
        r = {k: np.asarray(v, np.float32) for k, v in r.items()}
        g1 = r['g1'] * s
        g2 = r['g2'] * s
        out['res'].append({
            'w1': r['w1'] * g1[None, :], 'b1': r['b1'] * g1 + r['be1'],
            'w2': r['w2'] * g2[None, :], 'b2': r['b2'] * g2 + r['be2']})
    gc = p['gc'] * s
    out['wc2'] = p['wc2'] * gc[:, None]
    out['bc2'] = p['bc2'] + p['bec'] @ p['wc2']
    return out


def build_graph_layout(edge_index):
    """Assign every dst node to a (core, block, slot); lay out edges in
    (core, block, tile) order grouped by dst block."""
    import heapq
    src = np.asarray(edge_index[0], np.int64)
    dst = np.asarray(edge_index[1], np.int64)
    deg_raw = np.bincount(dst, minlength=N)
    dinv = (1.0 / np.sqrt(np.maximum(deg_raw + 1, 1.0))).astype(np.float32)
    rcnt = (1.0 / np.maximum(deg_raw, 1.0)).astype(np.float32)

    nblocks = NCORES * NBLK
    order = np.argsort(-deg_raw, kind='stable')
    blk_sum = np.zeros(nblocks, np.int64)
    blk_cnt = np.zeros(nblocks, np.int32)
    blk_members = [[] for _ in range(nblocks)]
    heap = [(0, b) for b in range(nblocks)]
    heapq.heapify(heap)
    for nidx in order:
        while True:
            _, b = heapq.heappop(heap)
            if blk_cnt[b] < BLK_DST:
                break
        blk_members[b].append(int(nidx))
        blk_cnt[b] += 1
        blk_sum[b] += deg_raw[nidx]
        if blk_cnt[b] < BLK_DST:
            heapq.heappush(heap, (int(blk_sum[b]), b))
    assert blk_sum.max() <= CHUNK_E, int(blk_sum.max())

    corder = np.argsort(-blk_sum, kind='stable')
    core_sum = np.zeros(NCORES, np.int64)
    core_nblk = np.zeros(NCORES, np.int32)
    core_blocks = [[] for _ in range(NCORES)]
    for b in corder:
        cands = [c for c in range(NCORES) if core_nblk[c] < NBLK]
        c = min(cands, key=lambda c: core_sum[c])
        core_blocks[c].append(int(b))
        core_nblk[c] += 1
        core_sum[c] += blk_sum[b]

    slot2node = -np.ones((NCORES, NSLOT), np.int64)
    node2core = np.zeros(N, np.int32)
    node2slot = np.zeros(N, np.int32)
    for c in range(NCORES):
        for bi, b in enumerate(core_blocks[c]):
            mem = blk_members[b]
            sl = np.arange(bi * BLK_DST, bi * BLK_DST + len(mem))
            slot2node[c, sl] = mem
            node2core[mem] = c
            node2slot[mem] = sl

    e_core = node2core[dst]
    e_slot = node2slot[dst]
    gsrc = np.zeros((NCORES, T * 128), np.int64)
    gloc = -np.ones((NCORES, T * 128), np.int64)   # dst slot % 64, -1 = pad
    gdst = np.zeros((NCORES, T * 128), np.int64)
    for c in range(NCORES):
        em = np.nonzero(e_core == c)[0]
        es, ed, eslot = src[em], dst[em], e_slot[em]
        blk = eslot // BLK_DST
        o2 = np.lexsort((eslot, blk))
        es, ed, eslot, blk = es[o2], ed[o2], eslot[o2], blk[o2]
        bounds = np.searchsorted(blk, np.arange(NBLK + 1))
        for bi in range(NBLK):
            lo, hi = int(bounds[bi]), int(bounds[bi + 1])
            k = hi - lo
            base = bi * CHUNK_E
            gsrc[c, base:base + k] = es[lo:hi]
            gdst[c, base:base + k] = ed[lo:hi]
            gloc[c, base:base + k] = eslot[lo:hi] % BLK_DST
    return dict(dinv=dinv, rcnt=rcnt, slot2node=slot2node, gsrc=gsrc,
                gdst=gdst, gloc=gloc, node2core=node2core, node2slot=node2slot)


def idx_image(gsrc):
    """int16 wrap-16 index image for dma_gather: [128, T*8]."""
    flat = gsrc.astype(np.int16).reshape(-1, 16)   # [T*8, 16]
    return np.ascontiguousarray(np.tile(flat.T, (8, 1)))


def build_masks(g):
    """masks bf16 [NCORES, 128, T, 128]; even chunk: [mask0 | mask_gcn],
    odd chunk: [mask_gcn | mask0] (so mask0 cols land at q*64)."""
    dinv = g['dinv']
    masks = np.zeros((NCORES, 128, T, 128), BF)
    for c in range(NCORES):
        gloc = g['gloc'][c]
        gsrc = g['gsrc'][c]
        e = np.nonzero(gloc >= 0)[0]
        t_ = e // 128
        p_ = e % 128
        chunk = e // CHUNK_E
        m0col = gloc[e]
        offs0 = np.where(chunk % 2 == 0, 0, 64)
        offsg = np.where(chunk % 2 == 0, 64, 0)
        m = np.zeros((128, T, 128), np.float32)
        m[p_, t_, m0col + offs0] = 1.0
        m[p_, t_, m0col + offsg] = dinv[gsrc[e]]
        masks[c] = m.astype(BF)
    return masks


# ======================================================================
# launch A program
# ======================================================================

def build_launch_a():
    nc = bacc.Bacc("TRN2", target_bir_lowering=False, debug=False,
                   num_devices=NCORES)
    xT = nc.dram_tensor("xT", [D_IN, NODEPAD], BF16, kind="ExternalInput").ap()
    w_in = nc.dram_tensor("w_in", [D_IN, D_H], BF16, kind="ExternalInput").ap()
    w_gat = nc.dram_tensor("w_gat", [D_H, D_H], BF16, kind="ExternalInput").ap()
    w_sr = nc.dram_tensor("w_sr", [D_H, D_H], BF16, kind="ExternalInput").ap()
    v_sd = nc.dram_tensor("v_sd", [D_H, 8], BF16, kind="ExternalInput").ap()
    b_in = nc.dram_tensor("b_in", [128, 4], F32, kind="ExternalInput").ap()
    hT = nc.dram_tensor("hT", [D_H, NODEPAD], F32, kind="ExternalOutput").ap()
    hgT = nc.dram_tensor("hgT", [D_H, NODEPAD], BF16, kind="ExternalOutput").ap()
    hsrT = nc.dram_tensor("hsrT", [D_H, NODEPAD], BF16, kind="ExternalOutput").ap()
    aT = nc.dram_tensor("aT", [8, NODEPAD], F32, kind="ExternalOutput").ap()

    KI = D_IN // 128   # 10
    KH = D_H // 128    # 4
    NCH = NODEPAD // 512  # 5

    with tile.TileContext(nc) as tc, ExitStack() as ctx:
        wpool = ctx.enter_context(tc.tile_pool(name="w", bufs=1))
        xpool = ctx.enter_context(tc.tile_pool(name="x", bufs=1))
        hpool = ctx.enter_context(tc.tile_pool(name="h", bufs=1))
        epool = ctx.enter_context(tc.tile_pool(name="ev", bufs=3))
        ppool = ctx.enter_context(tc.tile_pool(name="ps", bufs=4, space="PSUM"))

        w_in_sb = wpool.tile([128, KI, D_H], BF16)
        nc.sync.dma_start(w_in_sb[:], w_in.rearrange("(k p) m -> p k m", p=128))
        wg_sb = wpool.tile([128, KH, D_H], BF16)
        nc.sync.dma_start(wg_sb[:], w_gat.rearrange("(k p) m -> p k m", p=128))
        wsr_sb = wpool.tile([128, KH, D_H], BF16)
        nc.sync.dma_start(wsr_sb[:], w_sr.rearrange("(k p) m -> p k m", p=128))
        v_sb = wpool.tile([128, KH, 8], BF16)
        nc.sync.dma_start(v_sb[:], v_sd.rearrange("(k p) m -> p k m", p=128))
        bin_sb = wpool.tile([128, 4], F32)
        nc.sync.dma_start(bin_sb[:], b_in[:])

        xk = xpool.tile([128, KI, NODEPAD], BF16)
        nc.sync.dma_start(xk[:], xT.rearrange("(k p) n -> p k n", p=128))

        hbf = hpool.tile([128, KH, NODEPAD], BF16)

        # h = relu(x @ w_in + b_in)
        for m in range(KH):
            for nch in range(NCH):
                ps = ppool.tile([128, 512], F32, tag="ps")
                for k in range(KI):
                    nc.tensor.matmul(
                        ps[:],
                        lhsT=w_in_sb[:, k, m * 128:(m + 1) * 128],
                        rhs=xk[:, k, nch * 512:(nch + 1) * 512],
                        start=(k == 0), stop=(k == KI - 1))
                hf = epool.tile([128, 512], F32, tag="hf")
                nc.scalar.activation(hf[:], ps[:], AF.Relu,
                                     bias=bin_sb[:, m:m + 1])
                nc.sync.dma_start(
                    hT[m * 128:(m + 1) * 128, nch * 512:(nch + 1) * 512], hf[:])
                nc.vector.tensor_copy(
                    hbf[:, m, nch * 512:(nch + 1) * 512], hf[:])

        for (w_sb, outT) in ((wg_sb, hgT), (wsr_sb, hsrT)):
            for m in range(KH):
                for nch in range(NCH):
                    ps = ppool.tile([128, 512], F32, tag="ps")
                    for k in range(KH):
                        nc.tensor.matmul(
                            ps[:],
                            lhsT=w_sb[:, k, m * 128:(m + 1) * 128],
                            rhs=hbf[:, k, nch * 512:(nch + 1) * 512],
                            start=(k == 0), stop=(k == KH - 1))
                    ev = epool.tile([128, 512], BF16, tag="evb")
                    nc.scalar.activation(ev[:], ps[:], AF.Copy)
                    nc.sync.dma_start(
                        outT[m * 128:(m + 1) * 128,
                             nch * 512:(nch + 1) * 512], ev[:])
        for nch in range(NCH):
            ps = ppool.tile([8, 512], F32, tag="psa")
            for k in range(KH):
                nc.tensor.matmul(
                    ps[:], lhsT=v_sb[:, k, :],
                    rhs=hbf[:, k, nch * 512:(nch + 1) * 512],
                    start=(k == 0), stop=(k == KH - 1))
            ev = epool.tile([8, 512], F32, tag="eva")
            nc.scalar.activation(ev[:], ps[:], AF.Copy)
            nc.sync.dma_start(aT[:, nch * 512:(nch + 1) * 512], ev[:])
    nc.compile()
    return nc


# ======================================================================
# launch B program
# ======================================================================

def build_launch_b(debug=False):
    nc = bacc.Bacc("TRN2", target_bir_lowering=False, debug=False,
                   num_devices=NCORES)
    tbl = nc.dram_tensor("tbl", [N, TBL_W], BF16, kind="ExternalInput").ap()
    gidx = nc.dram_tensor("gidx", [128, T * 8], I16, kind="ExternalInput").ap()
    masks = nc.dram_tensor("masks", [128, T, 128], BF16, kind="ExternalInput").ap()
    adst = nc.dram_tensor("adst", [128, T, 4], F32, kind="ExternalInput").ap()
    aown = nc.dram_tensor("aown", [128, NPAIR, 8], F32, kind="ExternalInput").ap()
    hgown = nc.dram_tensor("hgown", [128, NPAIR, D_H], BF16, kind="ExternalInput").ap()
    hown2 = nc.dram_tensor("hown2", [128, NPAIR, D_H], BF16, kind="ExternalInput").ap()
    scal = nc.dram_tensor("scal", [128, NPAIR, 4], F32, kind="ExternalInput").ap()
    bgat = nc.dram_tensor("bgat", [128, D_H], F32, kind="ExternalInput").ap()
    hsrT = nc.dram_tensor("hsrT", [D_H, NSLOT], BF16, kind="ExternalInput").ap()
    wds = {}
    for nm in ("w_gcn", "w_sl", "r1w1", "r1w2", "r2w1", "r2w2"):
        wds[nm] = nc.dram_tensor(nm, [D_H, D_H], BF16, kind="ExternalInput").ap()
    wc1 = nc.dram_tensor("wc1", [D_H, 256], F32, kind="ExternalInput").ap()
    wc2 = nc.dram_tensor("wc2", [256, 1], F32, kind="ExternalInput").ap()
    bias = nc.dram_tensor("bias", [128, 32], F32, kind="ExternalInput").ap()
    logits = nc.dram_tensor("logits", [1, NSLOT], F32, kind="ExternalOutput").ap()
    dbg = {}
    if debug:
        for nm in ("d_x1", "d_x2p", "d_x3p"):
            dbg[nm] = nc.dram_tensor(nm, [128, NPAIR, D_H], F32,
                                     kind="ExternalOutput").ap()
        dbg["d_yT"] = nc.dram_tensor("d_yT", [D_H, NSLOT], F32,
                                     kind="ExternalOutput").ap()

    KH = D_H // 128   # 4
    NCH = NSLOT // 512  # 5

    with tile.TileContext(nc) as tc:
        # ---------- whole-kernel small statics ----------
        spool = tc.alloc_tile_pool(name="stat", bufs=1)
        aown_sb = spool.tile([128, NPAIR, 8], F32)
        nc.sync.dma_start(aown_sb[:], aown[:])
        scal_sb = spool.tile([128, NPAIR, 4], F32)
        nc.sync.dma_start(scal_sb[:], scal[:])
        bgat_sb = spool.tile([128, D_H], F32)
        nc.sync.dma_start(bgat_sb[:], bgat[:])
        bias_sb = spool.tile([128, 32], F32)
        nc.sync.dma_start(bias_sb[:], bias[:])
        idn = spool.tile([128, 128], BF16)
        cmasks.make_identity(nc, idn[:])

        # self-loop attention exp: exs = exp(lrelu(a_src + a_dst)) (own node)
        exs_sb = spool.tile([128, NPAIR, 4], F32)
        tmp_es = spool.tile([128, NPAIR, 4], F32)
        nc.vector.tensor_tensor(tmp_es[:], aown_sb[:, :, 0:4],
                                aown_sb[:, :, 4:8], ALU.add)
        nc.vector.scalar_tensor_tensor(tmp_es[:], tmp_es[:], LRELU, tmp_es[:],
                                       ALU.mult, ALU.max)
        nc.scalar.activation(exs_sb[:], tmp_es[:], AF.Exp)

        # accumulators (node-major)
        acc = tc.alloc_tile_pool(name="acc", bufs=1)
        x1pre = acc.tile([128, NPAIR, D_H], BF16)
        x2pre = acc.tile([128, NPAIR, D_H], BF16)
        x3pre = acc.tile([128, NPAIR, D_H], BF16)
        den = acc.tile([128, NPAIR, 4], F32)
        rden = acc.tile([128, NPAIR, 4], F32)

        # ---------- aggregation over edge chunks ----------
        lpool = tc.alloc_tile_pool(name="ldg", bufs=1)
        gidx_sb = lpool.tile([128, T * 8], I16)
        nc.sync.dma_start(gidx_sb[:], gidx[:])
        adst_sb = lpool.tile([128, T, 4], F32)
        nc.sync.dma_start(adst_sb[:], adst[:])
        hgown_sb = lpool.tile([128, NPAIR, D_H], BF16)
        nc.sync.dma_start(hgown_sb[:], hgown[:])
        hown2_sb = lpool.tile([128, NPAIR, D_H], BF16)
        nc.sync.dma_start(hown2_sb[:], hown2[:])

        gpool = tc.alloc_tile_pool(name="gath", bufs=2)
        mpool = tc.alloc_tile_pool(name="maskp", bufs=2)
        mgpool = tc.alloc_tile_pool(name="mg", bufs=3)
        expool = tc.alloc_tile_pool(name="ex", bufs=2)
        evpool = tc.alloc_tile_pool(name="aggev", bufs=3)
        ps_sg = tc.alloc_tile_pool(name="psg", bufs=2, space="PSUM")
        ps_g = tc.alloc_tile_pool(name="pgat", bufs=2, space="PSUM")
        ps_ex = tc.alloc_tile_pool(name="pex", bufs=2, space="PSUM")

        psum_g = None
        psum_ex = None
        for ci in range(NBLK):
            q = ci % 2
            pair = ci // 2
            gb = gpool.tile([128, TB, TBL_W], BF16, tag="gb", name="gb")
            nc.gpsimd.dma_gather(
                out_ap=gb[:],
                in_ap=tbl[:],
                idxs_ap=gidx_sb[:, ci * 64:(ci + 1) * 64],
                num_idxs=CHUNK_E,
                num_idxs_reg=CHUNK_E,
                elem_size=TBL_W,
            )
            mk = mpool.tile([128, TB, 128], BF16, tag="mk", name="mk")
            nc.sync.dma_start(mk[:], masks[:, ci * TB:(ci + 1) * TB, :])
            exf = expool.tile([128, TB, 4], F32, tag="exf", name="exf")
            nc.vector.tensor_tensor(
                exf[:], gb[:, :, EXTRA:EXTRA + 8].bitcast(F32),
                adst_sb[:, ci * TB:(ci + 1) * TB, :], ALU.add)
            nc.vector.scalar_tensor_tensor(exf[:], exf[:], LRELU, exf[:],
                                           ALU.mult, ALU.max)
            exb = expool.tile([128, TB, 4], BF16, tag="exb", name="exb")
            nc.scalar.activation(exb[:], exf[:], AF.Exp)

            if q == 0:
                psum_g = ps_g.tile([128, 512], F32, tag="pg", name="pg")
                psum_ex = ps_ex.tile([128, 4], F32, tag="pe", name="pe")
            psum_sg = ps_sg.tile([128, 512], F32, tag="psg", name="psg")

            for t in range(TB):
                mt = mk[:, t, :]
                mg = mgpool.tile([128, HEADS, 64], BF16, tag="mg", name="mg")
                nc.vector.tensor_tensor(
                    mg[:],
                    mt[:, q * 64:q * 64 + 64].unsqueeze(1)
                      .broadcast_to([128, HEADS, 64]),
                    exb[:, t, :].unsqueeze(2).broadcast_to([128, HEADS, 64]),
                    ALU.mult)
                nc.tensor.matmul(
                    psum_sg[:], lhsT=mt,
                    rhs=gb[:, t, 0:D_H],
                    start=(t == 0), stop=(t == TB - 1))
                nc.tensor.matmul(
                    psum_ex[q * 64:q * 64 + 64, :],
                    lhsT=mt[:, q * 64:q * 64 + 64],
                    rhs=exb[:, t, :],
                    start=(t == 0), stop=(t == TB - 1))
                for hh in range(HEADS):
                    # PSUM 'start' zeroes the whole partition-line of the
                    # bank, so only head 0 may issue it; heads 1-3 accumulate
                    # into the zeroed line.
                    nc.tensor.matmul(
                        psum_g[q * 64:q * 64 + 64, hh * CH:(hh + 1) * CH],
                        lhsT=mg[:, hh, :],
                        rhs=gb[:, t, D_H + hh * CH:D_H + (hh + 1) * CH],
                        start=(t == 0 and hh == 0), stop=(t == TB - 1),
                        skip_group_check=True)

            # per-chunk evictions from psum_sg:
            qs = slice(q * 64, q * 64 + 64)              # sage rows
            qg = slice((1 - q) * 64, (1 - q) * 64 + 64)  # gcn rows
            nc.vector.tensor_scalar(
                x3pre[qs, pair, :], psum_sg[qs, :],
                scal_sb[qs, pair, 0:1], None, op0=ALU.mult)
            t1 = evpool.tile([64, D_H], F32, tag="t1", name="t1")
            nc.vector.scalar_tensor_tensor(
                t1[:], hown2_sb[qg, pair, :], scal_sb[qg, pair, 1:2],
                psum_sg[qg, :], ALU.mult, ALU.add)
            nc.vector.tensor_scalar(
                x2pre[qg, pair, :], t1[:],
                scal_sb[qg, pair, 1:2], None, op0=ALU.mult)

            if q == 1:
                t2 = evpool.tile([128, HEADS, CH], F32, tag="t2", name="t2")
                nc.vector.tensor_tensor(
                    t2[:],
                    hgown_sb[:, pair, :].rearrange("p (h c) -> p h c", h=4),
                    exs_sb[:, pair, :].unsqueeze(2)
                        .broadcast_to([128, HEADS, CH]),
                    ALU.mult)
                nc.vector.tensor_tensor(
                    x1pre[:, pair, :].rearrange("p (h c) -> p h c", h=4),
                    t2[:], psum_g[:].rearrange("p (h c) -> p h c", h=4),
                    ALU.add)
                nc.vector.tensor_tensor(
                    den[:, pair, :], psum_ex[:], exs_sb[:, pair, :],
                    ALU.add)
        for p in (evpool, expool, mgpool, mpool, gpool, lpool):
            p.release()
        for p in (ps_ex, ps_g, ps_sg):
            p.release()

        # ---------- finalize x1 (batched, super-chunks of 5 pairs) -------
        fpool = tc.alloc_tile_pool(name="fin", bufs=1)
        nc.vector.reciprocal(rden[:], den[:])
        SC = 5
        for sc in range(NPAIR // SC):
            psl = slice(sc * SC, (sc + 1) * SC)
            z = fpool.tile([128, SC, HEADS, CH], F32, tag="z", name="z")
            nc.vector.tensor_tensor(
                z[:],
                x1pre[:, psl, :].rearrange("p n (h c) -> p n h c", h=4),
                rden[:, psl, :].unsqueeze(3).broadcast_to([128, SC, HEADS, CH]),
                ALU.mult)
            zf = z[:].rearrange("p n h c -> p n (h c)")
            nc.vector.tensor_tensor(
                zf, zf,
                bgat_sb[:].unsqueeze(1).broadcast_to([128, SC, D_H]),
                ALU.add)
            zn = fpool.tile([128, SC, D_H], F32, tag="zn", name="zn")
            nc.vector.tensor_scalar(zn[:], zf, 0.0, None, op0=ALU.min)
            en = fpool.tile([128, SC, D_H], F32, tag="en", name="en")
            nc.scalar.activation(en[:], zn[:], AF.Exp)
            # x1 = max(z,0) + exp(min(z,0))   (-1 folded into transpose step)
            nc.vector.scalar_tensor_tensor(
                x1pre[:, psl, :], zf, 0.0, en[:], ALU.max, ALU.add)
        if debug:
            for nm, src_t in (("d_x1", x1pre), ("d_x2p", x2pre),
                              ("d_x3p", x3pre)):
                for p in range(NPAIR):
                    cp = fpool.tile([128, D_H], F32, tag="dbgcp", name="cp")
                    nc.vector.tensor_copy(cp[:], src_t[:, p, :])
                    nc.sync.dma_start(dbg[nm][:, p, :], cp[:])
        fpool.release()

        # ---------- transposes to feature-major ----------
        ypool = tc.alloc_tile_pool(name="yp", bufs=1, side="right")
        yT = ypool.tile([128, KH, NSLOT], F32)
        xtp = tc.alloc_tile_pool(name="xtp", bufs=1, side="right")
        x2preT = xtp.tile([128, KH, NSLOT], BF16)
        x3preT = xtp.tile([128, KH, NSLOT], BF16)

        ps_tp = tc.alloc_tile_pool(name="pstp", bufs=2, space="PSUM")
        for p in range(NPAIR):
            for ccol in range(KH):
                cs = slice(ccol * 128, (ccol + 1) * 128)
                pt = ps_tp.tile([128, 128], BF16, tag="pt", name="pt")
                nc.tensor.transpose(pt[:], x1pre[:, p, cs], idn[:])
                nc.vector.tensor_scalar(
                    yT[:, ccol, p * 128:(p + 1) * 128], pt[:], -1.0, None,
                    op0=ALU.add)
                pt2 = ps_tp.tile([128, 128], BF16, tag="pt", name="pt2")
                nc.tensor.transpose(pt2[:], x2pre[:, p, cs], idn[:])
                # x2pre lanes are pair-swapped; swap back on write
                nc.vector.tensor_copy(
                    x2preT[:, ccol, p * 128 + 64:p * 128 + 128], pt2[:, 0:64])
                nc.vector.tensor_copy(
                    x2preT[:, ccol, p * 128:p * 128 + 64], pt2[:, 64:128])
                pt3 = ps_tp.tile([128, 128], BF16, tag="pt", name="pt3")
                nc.tensor.transpose(pt3[:], x3pre[:, p, cs], idn[:])
                nc.vector.tensor_copy(
                    x3preT[:, ccol, p * 128:(p + 1) * 128], pt3[:])
        ps_tp.release()
        acc.release()

        # ---------- dense tail (feature-major) ----------
        dwpool = tc.alloc_tile_pool(name="dw", bufs=1, side="right")
        w_sb = {}
        for nm in ("w_gcn", "w_sl", "r1w1", "r1w2", "r2w1", "r2w2"):
            w_sb[nm] = dwpool.tile([128, KH, D_H], BF16, tag="w_" + nm,
                                   name="w_" + nm)
            nc.sync.dma_start(w_sb[nm][:],
                              wds[nm].rearrange("(k p) m -> p k m", p=128))
        wc1_sb = dwpool.tile([128, KH, 256], F32)
        nc.sync.dma_start(wc1_sb[:], wc1.rearrange("(k p) m -> p k m", p=128))
        wc2_sb = dwpool.tile([128, 2, 1], F32)
        nc.sync.dma_start(wc2_sb[:], wc2.rearrange("(k p) m -> p k m", p=128))

        dpool = tc.alloc_tile_pool(name="dev", bufs=2, side="right")
        hpool = tc.alloc_tile_pool(name="hsr", bufs=2, side="right")
        ps_d = tc.alloc_tile_pool(name="psd", bufs=3, space="PSUM")
        ps_f = tc.alloc_tile_pool(name="psf", bufs=2, space="PSUM")

        def mm_layer(rhs_tile, w_tile, kdim, m, nch):
            ps = ps_d.tile([128, 512], F32, tag="psd", name="psd")
            for k in range(kdim):
                nc.tensor.matmul(
                    ps[:], lhsT=w_tile[:, k, m * 128:(m + 1) * 128],
                    rhs=rhs_tile[:, k, nch * 512:(nch + 1) * 512],
                    start=(k == 0), stop=(k == kdim - 1))
            return ps

        # yT += elu(x2preT@w_gcn + b_gcn) - 1 ; same for x3 (+hsr, b_sage)
        for m in range(KH):
            for nch in range(NCH):
                ncol = slice(nch * 512, (nch + 1) * 512)
                ps2 = mm_layer(x2preT, w_sb["w_gcn"], KH, m, nch)
                z = dpool.tile([128, 512], F32, tag="dz", name="dz")
                nc.scalar.activation(z[:], ps2[:], AF.Identity,
                                     bias=bias_sb[:, m:m + 1])
                zn = dpool.tile([128, 512], F32, tag="dzn", name="dzn")
                nc.vector.tensor_scalar(zn[:], z[:], 0.0, None, op0=ALU.min)
                en = dpool.tile([128, 512], F32, tag="den", name="den")
                nc.scalar.activation(en[:], zn[:], AF.Exp)
                nc.vector.scalar_tensor_tensor(z[:], z[:], 0.0, en[:],
                                               ALU.max, ALU.add)
                nc.vector.scalar_tensor_tensor(
                    yT[:, m, ncol], z[:], -1.0, yT[:, m, ncol],
                    ALU.add, ALU.add)

                ps3 = mm_layer(x3preT, w_sb["w_sl"], KH, m, nch)
                z3 = dpool.tile([128, 512], F32, tag="dz", name="dz3")
                nc.scalar.activation(z3[:], ps3[:], AF.Identity,
                                     bias=bias_sb[:, 4 + m:5 + m])
                hsr_t = hpool.tile([128, 512], BF16, tag="hsrt", name="hsrt")
                nc.sync.dma_start(hsr_t[:], hsrT[m * 128:(m + 1) * 128, ncol])
                nc.vector.tensor_tensor(z3[:], z3[:], hsr_t[:], ALU.add)
                zn3 = dpool.tile([128, 512], F32, tag="dzn", name="dzn3")
                nc.vector.tensor_scalar(zn3[:], z3[:], 0.0, None, op0=ALU.min)
                en3 = dpool.tile([128, 512], F32, tag="den", name="den3")
                nc.scalar.activation(en3[:], zn3[:], AF.Exp)
                nc.vector.scalar_tensor_tensor(z3[:], z3[:], 0.0, en3[:],
                                               ALU.max, ALU.add)
                nc.vector.scalar_tensor_tensor(
                    yT[:, m, ncol], z3[:], -1.0, yT[:, m, ncol],
                    ALU.add, ALU.add)
        hpool.release()

        if debug:
            nc.sync.dma_start(dbg["d_yT"].rearrange("(k p) n -> p k n", p=128),
                              yT[:])

        # residual blocks
        lpool2 = tc.alloc_tile_pool(name="late", bufs=1, side="right")
        yTb = lpool2.tile([128, KH, NSLOT], BF16)
        tT = lpool2.tile([128, KH, NSLOT], BF16)
        cT = lpool2.tile([128, 2, NSLOT], F32)
        nc.vector.tensor_copy(yTb[:], yT[:])
        for wn1, wn2, bo in (("r1w1", "r1w2", 8), ("r2w1", "r2w2", 16)):
            for m in range(KH):
                for nch in range(NCH):
                    ps = mm_layer(yTb, w_sb[wn1], KH, m, nch)
                    nc.scalar.activation(tT[:, m, nch * 512:(nch + 1) * 512],
                                         ps[:], AF.Relu,
                                         bias=bias_sb[:, bo + m:bo + 1 + m])
            for m in range(KH):
                for nch in range(NCH):
                    ncol = slice(nch * 512, (nch + 1) * 512)
                    ps = mm_layer(tT, w_sb[wn2], KH, m, nch)
                    z = dpool.tile([128, 512], F32, tag="dz", name="dzr")
                    nc.scalar.activation(z[:], ps[:], AF.Identity,
                                         bias=bias_sb[:, bo + 4 + m:bo + 5 + m])
                    nc.vector.tensor_tensor(z[:], z[:], yT[:, m, ncol],
                                            ALU.add)
                    nc.vector.tensor_scalar(yT[:, m, ncol], z[:], 0.0, None,
                                            op0=ALU.max)
            nc.vector.tensor_copy(yTb[:], yT[:])

        # classifier (f32 matmuls off the f32 yT for accuracy)
        for m in range(2):
            for nch in range(NCH):
                ps = mm_layer(yT, wc1_sb, KH, m, nch)
                nc.scalar.activation(cT[:, m, nch * 512:(nch + 1) * 512],
                                     ps[:], AF.Relu,
                                     bias=bias_sb[:, 24 + m:25 + m])
        for nch in range(NCH):
            ps = ps_f.tile([1, 512], F32, tag="psf", name="psf")
            for k in range(2):
                nc.tensor.matmul(ps[:], lhsT=wc2_sb[:, k, :],
                                 rhs=cT[:, k, nch * 512:(nch + 1) * 512],
                                 start=(k == 0), stop=(k == 1))
            ev = dpool.tile([1, 512], F32, tag="evf", name="evf")
            nc.scalar.activation(ev[:], ps[:], AF.Identity,
                                 bias=bias_sb[0:1, 26:27])
            nc.sync.dma_start(logits[:, nch * 512:(nch + 1) * 512], ev[:])
        for p in (ps_f, ps_d):
            p.release()
        for p in (lpool2, dpool, dwpool, xtp, ypool):
            p.release()
        spool.release()
    nc.compile()
    return nc


# ======================================================================
# cached SPMD runner (persistent jit; avoids per-call retrace) + timing
# ======================================================================

def _make_runner(nc):
    import jax
    from jax.sharding import Mesh, PartitionSpec, NamedSharding
    try:
        from jax.experimental.shard_map import shard_map
    except ImportError:
        from jax import shard_map
    from concourse import bass2jax as b2j
    from concourse import mybir as _mb

    b2j.install_neuronx_cc_hook()
    partition_name = (nc.partition_id_tensor.name
                      if nc.partition_id_tensor else None)
    in_names, out_names, out_avals, zero_outs = [], [], [], []
    for alloc in nc.m.functions[0].allocations:
        if not isinstance(alloc, _mb.MemoryLocationSet):
            continue
        name = alloc.memorylocations[0].name
        if alloc.kind == "ExternalInput":
            if name != partition_name:
                in_names.append(name)
        elif alloc.kind == "ExternalOutput":
            shape = tuple(alloc.tensor_shape)
            dtype = _mb.dt.np(alloc.dtype)
            out_names.append(name)
            out_avals.append(jax.core.ShapedArray(shape, dtype))
            zero_outs.append(np.zeros(shape, dtype))
    n_params = len(in_names)
    n_outs = len(out_avals)
    all_in_names = list(in_names) + list(out_names)
    if partition_name is not None:
        all_in_names.append(partition_name)

    def _body(*args):
        operands = list(args)
        if partition_name is not None:
            operands.append(b2j.partition_id_tensor())
        outs = b2j._bass_exec_p.bind(
            *operands,
            out_avals=tuple(out_avals),
            in_names=tuple(all_in_names),
            out_names=tuple(out_names),
            lowering_input_output_aliases=(),
            sim_require_finite=True,
            sim_require_nnan=True,
            nc=nc,
        )
        return tuple(outs)

    devices = jax.devices()[:NCORES]
    mesh = Mesh(np.asarray(devices), ("core",))
    in_specs = (PartitionSpec("core"),) * (n_params + n_outs)
    out_specs = (PartitionSpec("core"),) * n_outs
    fn = jax.jit(shard_map(_body, mesh=mesh, in_specs=in_specs,
                           out_specs=out_specs, check_rep=False),
                 keep_unused=True)
    shard = NamedSharding(mesh, PartitionSpec("core"))

    def run(in_maps, timeit=0):
        import time
        concat_in = [
            np.concatenate([np.asarray(in_maps[c][nm])
                            for c in range(NCORES)], axis=0)
            for nm in in_names]
        concat_zeros = [np.zeros((NCORES * z.shape[0], *z.shape[1:]), z.dtype)
                        for z in zero_outs]
        din = [jax.device_put(a, shard) for a in concat_in + concat_zeros]
        jax.block_until_ready(din)
        outs = fn(*din)
        jax.block_until_ready(outs)
        dt = None
        if timeit:
            t0 = time.perf_counter()
            last = None
            for _ in range(timeit):
                last = fn(*din)
            jax.block_until_ready(last)
            dt = (time.perf_counter() - t0) / timeit
        res = [
            {nm: np.asarray(outs[i]).reshape(NCORES, *out_avals[i].shape)[c]
             for i, nm in enumerate(out_names)}
            for c in range(NCORES)]
        return res, dt

    return run


def _get_runner(which, debug=False):
    key = ("runner", which, debug)
    if key not in _CACHE:
        nc_a, nc_b = _get_programs(debug=debug)
        _CACHE[key] = _make_runner(nc_a if which == "a" else nc_b)
    return _CACHE[key]


# ======================================================================
# host orchestration
# ======================================================================

def _get_programs(debug=False):
    key = ("progs", debug)
    if key not in _CACHE:
        _CACHE[key] = (build_launch_a(), build_launch_b(debug=debug))
    return _CACHE[key]


def prepare_a_inputs(x, fp):
    xt = np.asarray(x, np.float32).T.astype(BF)  # [1280, 20000]
    w_in = fp['w_in'].astype(BF)
    w_gat = fp['w_gat'].astype(BF)
    w_sr = fp['w_sage_r'].astype(BF)
    v_sd = np.concatenate([fp['v_src'], fp['v_dst']], 1).astype(BF)
    b_in = np.ascontiguousarray(fp['b_in'].reshape(4, 128).T)
    ins = []
    for c in range(NCORES):
        xc = np.zeros((D_IN, NODEPAD), BF)
        xc[:, :2500] = xt[:, c * 2500:(c + 1) * 2500]
        ins.append(dict(xT=xc, w_in=w_in, w_gat=w_gat, w_sr=w_sr,
                        v_sd=v_sd, b_in=b_in))
    return ins


def prepare_b_inputs(fp, g, h, hg, hsr, a_sd, masks_all):
    a_src = np.ascontiguousarray(a_sd[:, :4])
    a_dst = np.ascontiguousarray(a_sd[:, 4:])
    table = np.zeros((N, TBL_W), BF)
    table[:, :D_H] = h.astype(BF)
    table[:, D_H:2 * D_H] = hg
    table[:, EXTRA:EXTRA + 8] = a_src.astype('<f4').view(BF)

    bias = np.zeros((128, 32), np.float32)

    def putb(vec, col):
        v = np.asarray(vec, np.float32).reshape(-1, 128).T
        bias[:, col:col + v.shape[1]] = v
    putb(fp['b_gcn'], 0)
    putb(fp['b_sage'], 4)
    putb(fp['res'][0]['b1'], 8)
    putb(fp['res'][0]['b2'], 12)
    putb(fp['res'][1]['b1'], 16)
    putb(fp['res'][1]['b2'], 20)
    putb(fp['bc1'], 24)
    bias[0, 26] = fp['bc2'][0]

    wcommon = dict(
        w_gcn=fp['w_gcn'].astype(BF), w_sl=fp['w_sage_l'].astype(BF),
        r1w1=fp['res'][0]['w1'].astype(BF), r1w2=fp['res'][0]['w2'].astype(BF),
        r2w1=fp['res'][1]['w1'].astype(BF), r2w2=fp['res'][1]['w2'].astype(BF),
        wc1=np.ascontiguousarray(fp['wc1'], dtype=np.float32),
        wc2=np.ascontiguousarray(fp['wc2'], dtype=np.float32),
        bias=bias,
        bgat=np.ascontiguousarray(
            np.tile(fp['b_gat'][None, :], (128, 1)).astype(np.float32)),
        tbl=table)

    dinv, rcnt = g['dinv'], g['rcnt']
    ins = []
    slot = np.arange(NSLOT)
    b_ = slot // BLK_DST
    j_ = slot % BLK_DST
    p_ = b_ // 2
    q_ = b_ % 2
    Ln = q_ * 64 + j_
    Lf = (1 - q_) * 64 + j_
    for c in range(NCORES):
        s2n = g['slot2node'][c]
        valid = s2n >= 0
        s2nc = np.where(valid, s2n, 0)
        gloc_ = g['gloc'][c]
        gdst_ = g['gdst'][c]
        pad = gloc_ < 0

        gidx_img = idx_image(g['gsrc'][c])
        adst_e = np.where(pad[:, None], np.float32(PAD_ADST),
                          a_dst[np.where(pad, 0, gdst_)]).astype(np.float32)
        adst_img = np.ascontiguousarray(
            adst_e.reshape(T, 128, 4).transpose(1, 0, 2))

        aown = np.zeros((128, NPAIR, 8), np.float32)
        aown[Ln, p_, :] = np.where(valid[:, None],
                                   np.concatenate([a_src, a_dst], 1)[s2nc], 0)
        hgown = np.zeros((128, NPAIR, D_H), BF)
        hgown[Ln, p_, :] = np.where(valid[:, None], hg[s2nc].astype(np.float32),
                                    0).astype(BF)
        hown2 = np.zeros((128, NPAIR, D_H), BF)
        hown2[Lf, p_, :] = np.where(valid[:, None], h[s2nc], 0).astype(BF)
        scal = np.zeros((128, NPAIR, 4), np.float32)
        scal[Ln, p_, 0] = np.where(valid, rcnt[s2nc], 0)
        scal[Lf, p_, 1] = np.where(valid, dinv[s2nc], 0)
        hsr_slot = np.zeros((D_H, NSLOT), BF)
        hsr_slot[:, slot[valid]] = hsr[s2n[valid]].T

        d = dict(wcommon)
        d.update(gidx=gidx_img, masks=np.ascontiguousarray(masks_all[c]),
                 adst=adst_img, aown=aown, hgown=hgown, hown2=hown2,
                 scal=scal, hsrT=np.ascontiguousarray(hsr_slot))
        ins.append(d)
    return ins


def kernel(x, edge_index, params, debug=False, return_aux=False,
           timeit=0):
    x = np.asarray(x)
    edge_index = np.asarray(edge_index)
    nc_a, nc_b = _get_programs(debug=debug)
    fp = fold_params(params)
    g = build_graph_layout(edge_index)
    masks_all = build_masks(g)

    run_a = _get_runner("a", debug=debug)
    run_b = _get_runner("b", debug=debug)
    ins_a = prepare_a_inputs(x, fp)
    res_a_list, t_a = run_a(ins_a, timeit=timeit)
    h = np.zeros((N, D_H), np.float32)
    hg = np.zeros((N, D_H), BF)
    hsr = np.zeros((N, D_H), BF)
    a_sd = np.zeros((N, 8), np.float32)
    for c in range(NCORES):
        r = res_a_list[c]
        sl = slice(c * 2500, (c + 1) * 2500)
        h[sl] = np.asarray(r['hT'], np.float32).T[:2500]
        hg[sl] = np.asarray(r['hgT']).T[:2500].astype(BF)
        hsr[sl] = np.asarray(r['hsrT']).T[:2500].astype(BF)
        a_sd[sl] = np.asarray(r['aT'], np.float32).T[:2500]

    ins_b = prepare_b_inputs(fp, g, h, hg, hsr, a_sd, masks_all)
    res_b_list, t_b = run_b(ins_b, timeit=timeit)
    if timeit:
        print(f"launch A avg: {t_a*1e9:.0f} ns  launch B avg: {t_b*1e9:.0f} ns")
        print(f"HW exec time: {(t_a + t_b)*1e9:.0f} ns")

    out = np.zeros(N, np.float32)
    for c in range(NCORES):
        s2n = g['slot2node'][c]
        valid = s2n >= 0
        lg = np.asarray(res_b_list[c]['logits'], np.float32).reshape(-1)
        out[s2n[valid]] = lg[valid]
    if return_aux:
        class _R:
            pass
        ra = _R(); ra.results = res_a_list
        rb = _R(); rb.results = res_b_list
        return out, dict(res_a=ra, res_b=rb, g=g, fp=fp, h=h, hg=hg,
                         hsr=hsr, a_sd=a_sd)
    return out


# revision 16
# speedup vs baseline: 1.8580x; 1.8580x over previous
"""Trainium2 Bass kernel for nn_BindingSiteGNN (GAT+GCN+SAGE GNN).

Strategy (8 NeuronCores, SPMD):
  Launch A (row-parallel over nodes): input projection h = relu(bn(x@w_in)),
    plus hg = h@w_gat, hsr = h@w_sage_r and attention logits a = h@[v_src|v_dst],
    in feature-major layout. BatchNorms are folded into weights host-side.
  Host reshard: build a per-node gather table [h | hg | a_src] (bf16) plus
    per-edge mask/index data. Edges are grouped by destination into blocks of
    64 dst slots; dsts are bin-packed so every block holds <= 1024 edges and
    every core gets exactly 40 blocks (2560 dst slots).
  Launch B (edge-parallel by dst): per 1024-edge chunk, dma_gather pulls the
    table rows for the chunk's sources; PE aggregates per dst via mask
    matmuls accumulated in PSUM (SAGE/GCN: 0/1 and dinv-weighted host masks;
    GAT: mask0 * exp(attention) built on-chip); a per-dst epilogue applies
    GAT softmax denominators / GCN symmetric norm / SAGE mean, then the
    dense tail (residual MLP + classifier) runs feature-major.
  Host scatters per-core logits back to global node order.
"""
import sys

sys.path.insert(0, '/opt/trn_rl_repo')

from contextlib import ExitStack

import numpy as np
import ml_dtypes

import concourse.bacc as bacc
import concourse.tile as tile
import concourse.masks as cmasks
from concourse import mybir
from concourse.bass_utils import run_bass_kernel_spmd

BF = ml_dtypes.bfloat16
F32 = mybir.dt.float32
BF16 = mybir.dt.bfloat16
I16 = mybir.dt.int16
AF = mybir.ActivationFunctionType
ALU = mybir.AluOpType

N, E = 20000, 320000
D_IN, D_H = 1280, 512
HEADS, CH = 4, 128
BN_EPS = 1e-5
LRELU = 0.2
NCORES = 8

BLK_DST = 64            # dst slots per block
TB = 8                  # tiles (of 128 edges) per block
CHUNK_E = 128 * TB      # 1024 edges per chunk == one block
NBLK = 40               # blocks per core
NPAIR = NBLK // 2
T = NBLK * TB           # 320 tiles per core
NSLOT = NBLK * BLK_DST  # 2560 dst slots per core
NODEPAD = 2560          # padded node rows per core in launch A
TBL_W = 1152            # table row width (bf16 slots): h 512 | hg 512 | extras
EXTRA = 1024            # extras offset (slots)
PAD_ADST = -60.0        # pad edges: drives exp() to ~0

_CACHE = {}


# ======================================================================
# host-side preprocessing
# ======================================================================

def fold_params(params):
    s = np.float32(1.0 / np.sqrt(np.float32(1.0 + BN_EPS)))
    p = {k: np.asarray(v, np.float32) for k, v in params.items() if k != 'res'}
    out = {}
    g = p['bn_in_g'] * s
    out['w_in'] = p['w_in'] * g[None, :]
    out['b_in'] = p['b_in'] * g + p['bn_in_b']
    out['w_gat'] = p['w_gat']
    out['b_gat'] = p['b_gat']
    wg3 = p['w_gat'].reshape(D_H, HEADS, CH)
    out['v_src'] = np.einsum('dhc,hc->dh', wg3, p['att_src']).astype(np.float32)
    out['v_dst'] = np.einsum('dhc,hc->dh', wg3, p['att_dst']).astype(np.float32)
    for k in ('w_gcn', 'b_gcn', 'w_sage_l', 'w_sage_r', 'b_sage', 'wc1', 'bc1'):
        out[k] = p[k]
    out['res'] = []
    for r in params['res']:
        r = {k: np.asarray(v, np.float32) for k, v in r.items()}
        g1 = r['g1'] * s
        g2 = r['g2'] * s
        out['res'].append({
            'w1': r['w1'] * g1[None, :], 'b1': r['b1'] * g1 + r['be1'],
            'w2': r['w2'] * g2[None, :], 'b2': r['b2'] * g2 + r['be2']})
    gc = p['gc'] * s
    out['wc2'] = p['wc2'] * gc[:, None]
    out['bc2'] = p['bc2'] + p['bec'] @ p['wc2']
    return out


def build_graph_layout(edge_index):
    """Assign every dst node to a (core, block, slot); lay out edges in
    (core, block, tile) order grouped by dst block."""
    import heapq
    src = np.asarray(edge_index[0], np.int64)
    dst = np.asarray(edge_index[1], np.int64)
    deg_raw = np.bincount(dst, minlength=N)
    dinv = (1.0 / np.sqrt(np.maximum(deg_raw + 1, 1.0))).astype(np.float32)
    rcnt = (1.0 / np.maximum(deg_raw, 1.0)).astype(np.float32)

    nblocks = NCORES * NBLK
    order = np.argsort(-deg_raw, kind='stable')
    blk_sum = np.zeros(nblocks, np.int64)
    blk_cnt = np.zeros(nblocks, np.int32)
    blk_members = [[] for _ in range(nblocks)]
    heap = [(0, b) for b in range(nblocks)]
    heapq.heapify(heap)
    for nidx in order:
        while True:
            _, b = heapq.heappop(heap)
            if blk_cnt[b] < BLK_DST:
                break
        blk_members[b].append(int(nidx))
        blk_cnt[b] += 1
        blk_sum[b] += deg_raw[nidx]
        if blk_cnt[b] < BLK_DST:
            heapq.heappush(heap, (int(blk_sum[b]), b))
    assert blk_sum.max() <= CHUNK_E, int(blk_sum.max())

    corder = np.argsort(-blk_sum, kind='stable')
    core_sum = np.zeros(NCORES, np.int64)
    core_nblk = np.zeros(NCORES, np.int32)
    core_blocks = [[] for _ in range(NCORES)]
    for b in corder:
        cands = [c for c in range(NCORES) if core_nblk[c] < NBLK]
        c = min(cands, key=lambda c: core_sum[c])
        core_blocks[c].append(int(b))
        core_nblk[c] += 1
        core_sum[c] += blk_sum[b]

    slot2node = -np.ones((NCORES, NSLOT), np.int64)
    node2core = np.zeros(N, np.int32)
    node2slot = np.zeros(N, np.int32)
    for c in range(NCORES):
        for bi, b in enumerate(core_blocks[c]):
            mem = blk_members[b]
            sl = np.arange(bi * BLK_DST, bi * BLK_DST + len(mem))
            slot2node[c, sl] = mem
            node2core[mem] = c
            node2slot[mem] = sl

    e_core = node2core[dst]
    e_slot = node2slot[dst]
    gsrc = np.zeros((NCORES, T * 128), np.int64)
    gloc = -np.ones((NCORES, T * 128), np.int64)   # dst slot % 64, -1 = pad
    gdst = np.zeros((NCORES, T * 128), np.int64)
    for c in range(NCORES):
        em = np.nonzero(e_core == c)[0]
        es, ed, eslot = src[em], dst[em], e_slot[em]
        blk = eslot // BLK_DST
        o2 = np.lexsort((eslot, blk))
        es, ed, eslot, blk = es[o2], ed[o2], eslot[o2], blk[o2]
        bounds = np.searchsorted(blk, np.arange(NBLK + 1))
        for bi in range(NBLK):
            lo, hi = int(bounds[bi]), int(bounds[bi + 1])
            k = hi - lo
            base = bi * CHUNK_E
            gsrc[c, base:base + k] = es[lo:hi]
            gdst[c, base:base + k] = ed[lo:hi]
            gloc[c, base:base + k] = eslot[lo:hi] % BLK_DST
    return dict(dinv=dinv, rcnt=rcnt, slot2node=slot2node, gsrc=gsrc,
                gdst=gdst, gloc=gloc, node2core=node2core, node2slot=node2slot)


def idx_image(gsrc):
    """int16 wrap-16 index image for dma_gather: [128, T*8]."""
    flat = gsrc.astype(np.int16).reshape(-1, 16)   # [T*8, 16]
    return np.ascontiguousarray(np.tile(flat.T, (8, 1)))


def build_masks(g):
    """masks bf16 [NCORES, 128, T, 128]; even chunk: [mask0 | mask_gcn],
    odd chunk: [mask_gcn | mask0] (so mask0 cols land at q*64)."""
    dinv = g['dinv']
    masks = np.zeros((NCORES, 128, T, 128), BF)
    for c in range(NCORES):
        gloc = g['gloc'][c]
        gsrc = g['gsrc'][c]
        e = np.nonzero(gloc >= 0)[0]
        t_ = e // 128
        p_ = e % 128
        chunk = e // CHUNK_E
        m0col = gloc[e]
        offs0 = np.where(chunk % 2 == 0, 0, 64)
        offsg = np.where(chunk % 2 == 0, 64, 0)
        m = np.zeros((128, T, 128), np.float32)
        m[p_, t_, m0col + offs0] = 1.0
        m[p_, t_, m0col + offsg] = dinv[gsrc[e]]
        masks[c] = m.astype(BF)
    return masks


# ======================================================================
# launch A program
# ======================================================================

def build_launch_a():
    nc = bacc.Bacc("TRN2", target_bir_lowering=False, debug=False,
                   num_devices=NCORES)
    xT = nc.dram_tensor("xT", [D_IN, NODEPAD], BF16, kind="ExternalInput").ap()
    w_in = nc.dram_tensor("w_in", [D_IN, D_H], BF16, kind="ExternalInput").ap()
    w_gat = nc.dram_tensor("w_gat", [D_H, D_H], BF16, kind="ExternalInput").ap()
    w_sr = nc.dram_tensor("w_sr", [D_H, D_H], BF16, kind="ExternalInput").ap()
    v_sd = nc.dram_tensor("v_sd", [D_H, 8], BF16, kind="ExternalInput").ap()
    b_in = nc.dram_tensor("b_in", [128, 4], F32, kind="ExternalInput").ap()
    hT = nc.dram_tensor("hT", [D_H, NODEPAD], F32, kind="ExternalOutput").ap()
    hgT = nc.dram_tensor("hgT", [D_H, NODEPAD], BF16, kind="ExternalOutput").ap()
    hsrT = nc.dram_tensor("hsrT", [D_H, NODEPAD], BF16, kind="ExternalOutput").ap()
    aT = nc.dram_tensor("aT", [8, NODEPAD], F32, kind="ExternalOutput").ap()

    KI = D_IN // 128   # 10
    KH = D_H // 128    # 4
    NCH = NODEPAD // 512  # 5

    with tile.TileContext(nc) as tc, ExitStack() as ctx:
        wpool = ctx.enter_context(tc.tile_pool(name="w", bufs=1))
        xpool = ctx.enter_context(tc.tile_pool(name="x", bufs=1))
        hpool = ctx.enter_context(tc.tile_pool(name="h", bufs=1))
        epool = ctx.enter_context(tc.tile_pool(name="ev", bufs=3))
        ppool = ctx.enter_context(tc.tile_pool(name="ps", bufs=4, space="PSUM"))

        w_in_sb = wpool.tile([128, KI, D_H], BF16)
        nc.sync.dma_start(w_in_sb[:], w_in.rearrange("(k p) m -> p k m", p=128))
        wg_sb = wpool.tile([128, KH, D_H], BF16)
        nc.sync.dma_start(wg_sb[:], w_gat.rearrange("(k p) m -> p k m", p=128))
        wsr_sb = wpool.tile([128, KH, D_H], BF16)
        nc.sync.dma_start(wsr_sb[:], w_sr.rearrange("(k p) m -> p k m", p=128))
        v_sb = wpool.tile([128, KH, 8], BF16)
        nc.sync.dma_start(v_sb[:], v_sd.rearrange("(k p) m -> p k m", p=128))
        bin_sb = wpool.tile([128, 4], F32)
        nc.sync.dma_start(bin_sb[:], b_in[:])

        xk = xpool.tile([128, KI, NODEPAD], BF16)
        nc.sync.dma_start(xk[:], xT.rearrange("(k p) n -> p k n", p=128))

        hbf = hpool.tile([128, KH, NODEPAD], BF16)

        # h = relu(x @ w_in + b_in)
        for m in range(KH):
            for nch in range(NCH):
                ps = ppool.tile([128, 512], F32, tag="ps")
                for k in range(KI):
                    nc.tensor.matmul(
                        ps[:],
                        lhsT=w_in_sb[:, k, m * 128:(m + 1) * 128],
                        rhs=xk[:, k, nch * 512:(nch + 1) * 512],
                        start=(k == 0), stop=(k == KI - 1))
                hf = epool.tile([128, 512], F32, tag="hf")
                nc.scalar.activation(hf[:], ps[:], AF.Relu,
                                     bias=bin_sb[:, m:m + 1])
                nc.sync.dma_start(
                    hT[m * 128:(m + 1) * 128, nch * 512:(nch + 1) * 512], hf[:])
                nc.vector.tensor_copy(
                    hbf[:, m, nch * 512:(nch + 1) * 512], hf[:])

        for (w_sb, outT) in ((wg_sb, hgT), (wsr_sb, hsrT)):
            for m in range(KH):
                for nch in range(NCH):
                    ps = ppool.tile([128, 512], F32, tag="ps")
                    for k in range(KH):
                        nc.tensor.matmul(
                            ps[:],
                            lhsT=w_sb[:, k, m * 128:(m + 1) * 128],
                            rhs=hbf[:, k, nch * 512:(nch + 1) * 512],
                            start=(k == 0), stop=(k == KH - 1))
                    ev = epool.tile([128, 512], BF16, tag="evb")
                    nc.scalar.activation(ev[:], ps[:], AF.Copy)
                    nc.sync.dma_start(
                        outT[m * 128:(m + 1) * 128,
                             nch * 512:(nch + 1) * 512], ev[:])
        for nch in range(NCH):
            ps = ppool.tile([8, 512], F32, tag="psa")
            for k in range(KH):
                nc.tensor.matmul(
                    ps[:], lhsT=v_sb[:, k, :],
                    rhs=hbf[:, k, nch * 512:(nch + 1) * 512],
                    start=(k == 0), stop=(k == KH - 1))
            ev = epool.tile([8, 512], F32, tag="eva")
            nc.scalar.activation(ev[:], ps[:], AF.Copy)
            nc.sync.dma_start(aT[:, nch * 512:(nch + 1) * 512], ev[:])
    nc.compile()
    return nc


# ======================================================================
# launch B program
# ======================================================================

def build_launch_b(debug=False):
    nc = bacc.Bacc("TRN2", target_bir_lowering=False, debug=False,
                   num_devices=NCORES)
    tbl = nc.dram_tensor("tbl", [N, TBL_W], BF16, kind="ExternalInput").ap()
    gidx = nc.dram_tensor("gidx", [128, T * 8], I16, kind="ExternalInput").ap()
    masks = nc.dram_tensor("masks", [128, T, 128], BF16, kind="ExternalInput").ap()
    adst = nc.dram_tensor("adst", [128, T, 4], F32, kind="ExternalInput").ap()
    aown = nc.dram_tensor("aown", [128, NPAIR, 8], F32, kind="ExternalInput").ap()
    hgown = nc.dram_tensor("hgown", [128, NPAIR, D_H], BF16, kind="ExternalInput").ap()
    hown2 = nc.dram_tensor("hown2", [128, NPAIR, D_H], BF16, kind="ExternalInput").ap()
    scal = nc.dram_tensor("scal", [128, NPAIR, 4], F32, kind="ExternalInput").ap()
    bgat = nc.dram_tensor("bgat", [128, D_H], F32, kind="ExternalInput").ap()
    hsrT = nc.dram_tensor("hsrT", [D_H, NSLOT], BF16, kind="ExternalInput").ap()
    wds = {}
    for nm in ("w_gcn", "w_sl", "r1w1", "r1w2", "r2w1", "r2w2"):
        wds[nm] = nc.dram_tensor(nm, [D_H, D_H], BF16, kind="ExternalInput").ap()
    wc1 = nc.dram_tensor("wc1", [D_H, 256], F32, kind="ExternalInput").ap()
    wc2 = nc.dram_tensor("wc2", [256, 1], F32, kind="ExternalInput").ap()
    bias = nc.dram_tensor("bias", [128, 32], F32, kind="ExternalInput").ap()
    logits = nc.dram_tensor("logits", [1, NSLOT], F32, kind="ExternalOutput").ap()
    dbg = {}
    if debug:
        for nm in ("d_x1", "d_x2p", "d_x3p"):
            dbg[nm] = nc.dram_tensor(nm, [128, NPAIR, D_H], F32,
                                     kind="ExternalOutput").ap()
        dbg["d_yT"] = nc.dram_tensor("d_yT", [D_H, NSLOT], F32,
                                     kind="ExternalOutput").ap()

    KH = D_H // 128   # 4
    NCH = NSLOT // 512  # 5

    with tile.TileContext(nc) as tc:
        # ---------- whole-kernel small statics (left stack bottom) -------
        spool = tc.alloc_tile_pool(name="stat", bufs=1)
        aown_sb = spool.tile([128, NPAIR, 8], F32)
        nc.sync.dma_start(aown_sb[:], aown[:])
        scal_sb = spool.tile([128, NPAIR, 4], F32)
        nc.sync.dma_start(scal_sb[:], scal[:])
        bgat_sb = spool.tile([128, D_H], F32)
        nc.sync.dma_start(bgat_sb[:], bgat[:])
        bias_sb = spool.tile([128, 32], F32)
        nc.sync.dma_start(bias_sb[:], bias[:])
        idn = spool.tile([128, 128], BF16)
        cmasks.make_identity(nc, idn[:])
        gidx_sb = spool.tile([128, T * 8], I16)
        nc.sync.dma_start(gidx_sb[:], gidx[:])

        exs_sb = spool.tile([128, NPAIR, 4], F32)
        tmp_es = spool.tile([128, NPAIR, 4], F32)
        nc.vector.tensor_tensor(tmp_es[:], aown_sb[:, :, 0:4],
                                aown_sb[:, :, 4:8], ALU.add)
        nc.vector.scalar_tensor_tensor(tmp_es[:], tmp_es[:], LRELU, tmp_es[:],
                                       ALU.mult, ALU.max)
        nc.scalar.activation(exs_sb[:], tmp_es[:], AF.Exp)

        # ---------- right stack: long-lived dense-phase tensors ----------
        ypool = tc.alloc_tile_pool(name="yp", bufs=1, side="right")
        yT = ypool.tile([128, KH, NSLOT], F32)
        dpool = tc.alloc_tile_pool(name="dev", bufs=2, side="right")
        xtp = tc.alloc_tile_pool(name="xtp", bufs=1, side="right")
        x2preT = xtp.tile([128, KH, NSLOT], BF16)
        x3preT = xtp.tile([128, KH, NSLOT], BF16)
        dwe = tc.alloc_tile_pool(name="dwe", bufs=1, side="right")
        w_sb = {}
        for nm in ("w_gcn", "w_sl"):
            w_sb[nm] = dwe.tile([128, KH, D_H], BF16, tag="w_" + nm,
                                name="w_" + nm)
            nc.sync.dma_start(w_sb[nm][:],
                              wds[nm].rearrange("(k p) m -> p k m", p=128))
        hpool = tc.alloc_tile_pool(name="hsr", bufs=2, side="right")

        # ---------- agg-phase pools ----------
        gpool = tc.alloc_tile_pool(name="gath", bufs=2)
        mpool = tc.alloc_tile_pool(name="maskp", bufs=2)
        mgpool = tc.alloc_tile_pool(name="mg", bufs=2)
        expool = tc.alloc_tile_pool(name="ex", bufs=2)
        adpool = tc.alloc_tile_pool(name="adp", bufs=2)
        prpool = tc.alloc_tile_pool(name="pair", bufs=2)
        fipool = tc.alloc_tile_pool(name="fin", bufs=1)
        evpool = tc.alloc_tile_pool(name="aggev", bufs=2)
        ps_sg = tc.alloc_tile_pool(name="psg", bufs=2, space="PSUM")
        ps_g = tc.alloc_tile_pool(name="pgat", bufs=2, space="PSUM")
        ps_ex = tc.alloc_tile_pool(name="pex", bufs=2, space="PSUM")
        ps_mix = tc.alloc_tile_pool(name="pmix", bufs=2, space="PSUM")

        def mm_layer(rhs_tile, w_tile, kdim, m, nch):
            ps = ps_mix.tile([128, 512], F32, tag="mix", name="psd")
            for k in range(kdim):
                nc.tensor.matmul(
                    ps[:], lhsT=w_tile[:, k, m * 128:(m + 1) * 128],
                    rhs=rhs_tile[:, k, nch * 512:(nch + 1) * 512],
                    start=(k == 0), stop=(k == kdim - 1))
            return ps

        def elu_into_yT(ps, m, ncol, bias_ap, extra=None):
            z = dpool.tile([128, 512], F32, tag="dz", name="dz")
            nc.scalar.activation(z[:], ps[:], AF.Identity, bias=bias_ap)
            if extra is not None:
                nc.vector.tensor_tensor(z[:], z[:], extra, ALU.add)
            zn = dpool.tile([128, 512], F32, tag="dzn", name="dzn")
            nc.vector.tensor_scalar(zn[:], z[:], 0.0, None, op0=ALU.min)
            en = dpool.tile([128, 512], F32, tag="den", name="den")
            nc.scalar.activation(en[:], zn[:], AF.Exp)
            nc.vector.scalar_tensor_tensor(z[:], z[:], 0.0, en[:],
                                           ALU.max, ALU.add)
            nc.vector.scalar_tensor_tensor(
                yT[:, m, ncol], z[:], -1.0, yT[:, m, ncol], ALU.add, ALU.add)

        psum_g = None
        psum_ex = None
        x1p = x2p = x3p = None
        fdbg = tc.alloc_tile_pool(name="fdbg", bufs=2) if debug else None
        for ci in range(NBLK):
            q = ci % 2
            pair = ci // 2
            gb = gpool.tile([128, TB, TBL_W], BF16, tag="gb", name="gb")
            nc.gpsimd.dma_gather(
                out_ap=gb[:],
                in_ap=tbl[:],
                idxs_ap=gidx_sb[:, ci * 64:(ci + 1) * 64],
                num_idxs=CHUNK_E,
                num_idxs_reg=CHUNK_E,
                elem_size=TBL_W,
            )
            mk = mpool.tile([128, TB, 128], BF16, tag="mk", name="mk")
            nc.sync.dma_start(mk[:], masks[:, ci * TB:(ci + 1) * TB, :])
            ad = adpool.tile([128, TB, 4], F32, tag="ad", name="ad")
            nc.sync.dma_start(ad[:], adst[:, ci * TB:(ci + 1) * TB, :])
            exf = expool.tile([128, TB, 4], F32, tag="exf", name="exf")
            nc.vector.tensor_tensor(
                exf[:], gb[:, :, EXTRA:EXTRA + 8].bitcast(F32), ad[:], ALU.add)
            nc.vector.scalar_tensor_tensor(exf[:], exf[:], LRELU, exf[:],
                                           ALU.mult, ALU.max)
            exb = expool.tile([128, TB, 4], BF16, tag="exb", name="exb")
            nc.scalar.activation(exb[:], exf[:], AF.Exp)
            # gat masks for the whole chunk in one op: [128, TB, 4, 64]
            mg = mgpool.tile([128, TB, HEADS, 64], BF16, tag="mg", name="mg")
            nc.vector.tensor_tensor(
                mg[:],
                mk[:, :, q * 64:q * 64 + 64].unsqueeze(2)
                  .broadcast_to([128, TB, HEADS, 64]),
                exb[:].unsqueeze(3).broadcast_to([128, TB, HEADS, 64]),
                ALU.mult)

            if q == 0:
                psum_g = ps_g.tile([128, 512], F32, tag="pg", name="pg")
                psum_ex = ps_ex.tile([128, 4], F32, tag="pe", name="pe")
                x1p = prpool.tile([128, D_H], BF16, tag="x1p", name="x1p")
                x2p = prpool.tile([128, D_H], BF16, tag="x2p", name="x2p")
                x3p = prpool.tile([128, D_H], BF16, tag="x3p", name="x3p")
            psum_sg = ps_sg.tile([128, 512], F32, tag="psg", name="psg")

            for t in range(TB):
                mt = mk[:, t, :]
                nc.tensor.matmul(
                    psum_sg[:], lhsT=mt, rhs=gb[:, t, 0:D_H],
                    start=(t == 0), stop=(t == TB - 1))
                nc.tensor.matmul(
                    psum_ex[q * 64:q * 64 + 64, :],
                    lhsT=mt[:, q * 64:q * 64 + 64],
                    rhs=exb[:, t, :],
                    start=(t == 0), stop=(t == TB - 1))
                for hh in range(HEADS):
                    # PSUM start zeroes the whole partition-line of the bank:
                    # only head 0 issues it.
                    nc.tensor.matmul(
                        psum_g[q * 64:q * 64 + 64, hh * CH:(hh + 1) * CH],
                        lhsT=mg[:, t, hh, :],
                        rhs=gb[:, t, D_H + hh * CH:D_H + (hh + 1) * CH],
                        start=(t == 0 and hh == 0), stop=(t == TB - 1),
                        skip_group_check=True)

            # per-chunk evictions from psum_sg (sage rows q*64, gcn (1-q)*64)
            qs = slice(q * 64, q * 64 + 64)
            qg = slice((1 - q) * 64, (1 - q) * 64 + 64)
            nc.vector.tensor_scalar(
                x3p[qs, :], psum_sg[qs, :],
                scal_sb[qs, pair, 0:1], None, op0=ALU.mult)
            ho = hpool.tile([128, D_H], BF16, tag="ho", name="ho")
            nc.sync.dma_start(ho[qg, :], hown2[qg, pair, :])
            t1 = evpool.tile([64, D_H], F32, tag="t1", name="t1")
            nc.vector.scalar_tensor_tensor(
                t1[:], ho[qg, :], scal_sb[qg, pair, 1:2],
                psum_sg[qg, :], ALU.mult, ALU.add)
            nc.vector.tensor_scalar(
                x2p[qg, :], t1[:], scal_sb[qg, pair, 1:2], None, op0=ALU.mult)

            if q == 1:
                # ---- pair epilogue: x1 assembly + finalize ----
                hg_o = hpool.tile([128, D_H], BF16, tag="hgo", name="hgo")
                nc.sync.dma_start(hg_o[:], hgown[:, pair, :])
                t2 = evpool.tile([128, HEADS, CH], F32, tag="t2", name="t2")
                nc.vector.tensor_tensor(
                    t2[:], hg_o[:].rearrange("p (h c) -> p h c", h=4),
                    exs_sb[:, pair, :].unsqueeze(2)
                        .broadcast_to([128, HEADS, CH]),
                    ALU.mult)
                den = evpool.tile([128, 4], F32, tag="pden", name="pden")
                nc.vector.tensor_tensor(den[:], psum_ex[:],
                                        exs_sb[:, pair, :], ALU.add)
                rden = evpool.tile([128, 4], F32, tag="prden", name="prden")
                nc.vector.reciprocal(rden[:], den[:])
                z = fipool.tile([128, HEADS, CH], F32, tag="z", name="z")
                nc.vector.tensor_tensor(
                    z[:], t2[:], psum_g[:].rearrange("p (h c) -> p h c", h=4),
                    ALU.add)
                nc.vector.tensor_tensor(
                    z[:], z[:],
                    rden[:].unsqueeze(2).broadcast_to([128, HEADS, CH]),
                    ALU.mult)
                zf = z[:].rearrange("p h c -> p (h c)")
                nc.vector.tensor_tensor(zf, zf, bgat_sb[:], ALU.add)
                zn = fipool.tile([128, D_H], F32, tag="zn", name="zn")
                nc.vector.tensor_scalar(zn[:], zf, 0.0, None, op0=ALU.min)
                en = fipool.tile([128, D_H], F32, tag="en", name="en")
                nc.scalar.activation(en[:], zn[:], AF.Exp)
                nc.vector.scalar_tensor_tensor(
                    x1p[:], zf, 0.0, en[:], ALU.max, ALU.add)
                if debug:
                    for nm, src_t in (("d_x1", x1p), ("d_x2p", x2p),
                                      ("d_x3p", x3p)):
                        cp = fdbg.tile([128, D_H], F32, tag="dbgcp", name="cp")
                        nc.vector.tensor_copy(cp[:], src_t[:])
                        nc.sync.dma_start(dbg[nm][:, pair, :], cp[:])

                # ---- transposes into feature-major targets ----
                for ccol in range(KH):
                    cs = slice(ccol * 128, (ccol + 1) * 128)
                    pcol = slice(pair * 128, (pair + 1) * 128)
                    pt = ps_mix.tile([128, 512], BF16, tag="mix", name="pt")
                    nc.tensor.transpose(pt[:, 0:128], x1p[:, cs], idn[:])
                    nc.vector.tensor_scalar(
                        yT[:, ccol, pcol], pt[:, 0:128], -1.0, None,
                        op0=ALU.add)
                    pt2 = ps_mix.tile([128, 512], BF16, tag="mix", name="pt2")
                    nc.tensor.transpose(pt2[:, 0:128], x2p[:, cs], idn[:])
                    nc.vector.tensor_copy(
                        x2preT[:, ccol, pair * 128 + 64:pair * 128 + 128],
                        pt2[:, 0:64])
                    nc.vector.tensor_copy(
                        x2preT[:, ccol, pair * 128:pair * 128 + 64],
                        pt2[:, 64:128])
                    pt3 = ps_mix.tile([128, 512], BF16, tag="mix", name="pt3")
                    nc.tensor.transpose(pt3[:, 0:128], x3p[:, cs], idn[:])
                    nc.vector.tensor_copy(x3preT[:, ccol, pcol], pt3[:, 0:128])

                # ---- x2/x3 dense for a completed 512-node column chunk ----
                if pair % 4 == 3:
                    nch = pair // 4
                    ncol = slice(nch * 512, (nch + 1) * 512)
                    for m in range(KH):
                        ps2 = mm_layer(x2preT, w_sb["w_gcn"], KH, m, nch)
                        elu_into_yT(ps2, m, ncol, bias_sb[:, m:m + 1])
                        ps3 = mm_layer(x3preT, w_sb["w_sl"], KH, m, nch)
                        hsr_t = hpool.tile([128, 512], BF16, tag="hsrt",
                                           name="hsrt")
                        nc.sync.dma_start(hsr_t[:],
                                          hsrT[m * 128:(m + 1) * 128, ncol])
                        elu_into_yT(ps3, m, ncol, bias_sb[:, 4 + m:5 + m],
                                    extra=hsr_t[:])

        for p in (ps_mix, ps_ex, ps_g, ps_sg):
            p.release()
        if fdbg is not None:
            fdbg.release()
        for p in (evpool, fipool, prpool, adpool, expool, mgpool, mpool,
                  gpool):
            p.release()
        hpool.release()
        dwe.release()
        xtp.release()

        if debug:
            nc.sync.dma_start(dbg["d_yT"].rearrange("(k p) n -> p k n", p=128),
                              yT[:])

        # ---------- tail: residual blocks + classifier ----------
        dwl = tc.alloc_tile_pool(name="dwl", bufs=1, side="right")
        w2_sb = {}
        for nm in ("r1w1", "r1w2", "r2w1", "r2w2"):
            w2_sb[nm] = dwl.tile([128, KH, D_H], BF16, tag="w_" + nm,
                                 name="w_" + nm)
            nc.sync.dma_start(w2_sb[nm][:],
                              wds[nm].rearrange("(k p) m -> p k m", p=128))
        wc1_sb = dwl.tile([128, KH, 256], F32)
        nc.sync.dma_start(wc1_sb[:], wc1.rearrange("(k p) m -> p k m", p=128))
        wc2_sb = dwl.tile([128, 2, 1], F32)
        nc.sync.dma_start(wc2_sb[:], wc2.rearrange("(k p) m -> p k m", p=128))
        lpool2 = tc.alloc_tile_pool(name="late", bufs=1, side="right")
        yTb = lpool2.tile([128, KH, NSLOT], BF16)
        tT = lpool2.tile([128, KH, NSLOT], BF16)
        cT = lpool2.tile([128, 2, NSLOT], F32)
        ps_t = tc.alloc_tile_pool(name="pst", bufs=3, space="PSUM")
        ps_f = tc.alloc_tile_pool(name="psf", bufs=2, space="PSUM")

        def mm_tail(rhs_tile, w_tile, kdim, m, nch):
            ps = ps_t.tile([128, 512], F32, tag="pst", name="pst")
            for k in range(kdim):
                nc.tensor.matmul(
                    ps[:], lhsT=w_tile[:, k, m * 128:(m + 1) * 128],
                    rhs=rhs_tile[:, k, nch * 512:(nch + 1) * 512],
                    start=(k == 0), stop=(k == kdim - 1))
            return ps

        nc.vector.tensor_copy(yTb[:], yT[:])
        for wn1, wn2, bo in (("r1w1", "r1w2", 8), ("r2w1", "r2w2", 16)):
            for m in range(KH):
                for nch in range(NCH):
                    ps = mm_tail(yTb, w2_sb[wn1], KH, m, nch)
                    nc.scalar.activation(tT[:, m, nch * 512:(nch + 1) * 512],
                                         ps[:], AF.Relu,
                                         bias=bias_sb[:, bo + m:bo + 1 + m])
            for m in range(KH):
                for nch in range(NCH):
                    ncol = slice(nch * 512, (nch + 1) * 512)
                    ps = mm_tail(tT, w2_sb[wn2], KH, m, nch)
                    z = dpool.tile([128, 512], F32, tag="dz", name="dzr")
                    nc.scalar.activation(z[:], ps[:], AF.Identity,
                                         bias=bias_sb[:, bo + 4 + m:bo + 5 + m])
                    nc.vector.tensor_tensor(z[:], z[:], yT[:, m, ncol],
                                            ALU.add)
                    nc.vector.tensor_scalar(yT[:, m, ncol], z[:], 0.0, None,
                                            op0=ALU.max)
            nc.vector.tensor_copy(yTb[:], yT[:])

        # classifier in f32 straight off the f32 yT
        for m in range(2):
            for nch in range(NCH):
                ps = mm_tail(yT, wc1_sb, KH, m, nch)
                nc.scalar.activation(cT[:, m, nch * 512:(nch + 1) * 512],
                                     ps[:], AF.Relu,
                                     bias=bias_sb[:, 24 + m:25 + m])
        for nch in range(NCH):
            ps = ps_f.tile([1, 512], F32, tag="psf", name="psf")
            for k in range(2):
                nc.tensor.matmul(ps[:], lhsT=wc2_sb[:, k, :],
                                 rhs=cT[:, k, nch * 512:(nch + 1) * 512],
                                 start=(k == 0), stop=(k == 1))
            ev = dpool.tile([1, 512], F32, tag="evf", name="evf")
            nc.scalar.activation(ev[:], ps[:], AF.Identity,
                                 bias=bias_sb[0:1, 26:27])
            nc.sync.dma_start(logits[:, nch * 512:(nch + 1) * 512], ev[:])

        for p in (ps_f, ps_t):
            p.release()
        for p in (lpool2, dwl, dpool, ypool):
            p.release()
        spool.release()
    nc.compile()
    return nc


# ======================================================================
# cached SPMD runner (persistent jit; avoids per-call retrace) + timing
# ======================================================================

def _make_runner(nc, n_cores=NCORES):
    import jax
    from jax.sharding import Mesh, PartitionSpec, NamedSharding
    try:
        from jax.experimental.shard_map import shard_map
    except ImportError:
        from jax import shard_map
    from concourse import bass2jax as b2j
    from concourse import mybir as _mb

    b2j.install_neuronx_cc_hook()
    partition_name = (nc.partition_id_tensor.name
                      if nc.partition_id_tensor else None)
    in_names, out_names, out_avals, zero_outs = [], [], [], []
    for alloc in nc.m.functions[0].allocations:
        if not isinstance(alloc, _mb.MemoryLocationSet):
            continue
        name = alloc.memorylocations[0].name
        if alloc.kind == "ExternalInput":
            if name != partition_name:
                in_names.append(name)
        elif alloc.kind == "ExternalOutput":
            shape = tuple(alloc.tensor_shape)
            dtype = _mb.dt.np(alloc.dtype)
            out_names.append(name)
            out_avals.append(jax.core.ShapedArray(shape, dtype))
            zero_outs.append(np.zeros(shape, dtype))
    n_params = len(in_names)
    n_outs = len(out_avals)
    all_in_names = list(in_names) + list(out_names)
    if partition_name is not None:
        all_in_names.append(partition_name)

    def _body(*args):
        operands = list(args)
        if partition_name is not None:
            operands.append(b2j.partition_id_tensor())
        outs = b2j._bass_exec_p.bind(
            *operands,
            out_avals=tuple(out_avals),
            in_names=tuple(all_in_names),
            out_names=tuple(out_names),
            lowering_input_output_aliases=(),
            sim_require_finite=True,
            sim_require_nnan=True,
            nc=nc,
        )
        return tuple(outs)

    devices = jax.devices()[:n_cores]
    mesh = Mesh(np.asarray(devices), ("core",))
    in_specs = (PartitionSpec("core"),) * (n_params + n_outs)
    out_specs = (PartitionSpec("core"),) * n_outs
    fn = jax.jit(shard_map(_body, mesh=mesh, in_specs=in_specs,
                           out_specs=out_specs, check_rep=False),
                 keep_unused=True)
    shard = NamedSharding(mesh, PartitionSpec("core"))

    def run(in_maps, timeit=0):
        import time
        concat_in = [
            np.concatenate([np.asarray(in_maps[c][nm])
                            for c in range(n_cores)], axis=0)
            for nm in in_names]
        concat_zeros = [np.zeros((n_cores * z.shape[0], *z.shape[1:]), z.dtype)
                        for z in zero_outs]
        din = [jax.device_put(a, shard) for a in concat_in + concat_zeros]
        jax.block_until_ready(din)
        outs = fn(*din)
        jax.block_until_ready(outs)
        dt = None
        if timeit:
            t0 = time.perf_counter()
            last = None
            for _ in range(timeit):
                last = fn(*din)
            jax.block_until_ready(last)
            dt = (time.perf_counter() - t0) / timeit
        res = [
            {nm: np.asarray(outs[i]).reshape(n_cores, *out_avals[i].shape)[c]
             for i, nm in enumerate(out_names)}
            for c in range(n_cores)]
        return res, dt

    return run


def _get_runner(which, debug=False):
    key = ("runner", which, debug)
    if key not in _CACHE:
        nc_a, nc_b = _get_programs(debug=debug)
        _CACHE[key] = _make_runner(nc_a if which == "a" else nc_b)
    return _CACHE[key]


# ======================================================================
# host orchestration
# ======================================================================

def _get_programs(debug=False):
    key = ("progs", debug)
    if key not in _CACHE:
        _CACHE[key] = (build_launch_a(), build_launch_b(debug=debug))
    return _CACHE[key]


def prepare_a_inputs(x, fp):
    xt = np.asarray(x, np.float32).T.astype(BF)  # [1280, 20000]
    w_in = fp['w_in'].astype(BF)
    w_gat = fp['w_gat'].astype(BF)
    w_sr = fp['w_sage_r'].astype(BF)
    v_sd = np.concatenate([fp['v_src'], fp['v_dst']], 1).astype(BF)
    b_in = np.ascontiguousarray(fp['b_in'].reshape(4, 128).T)
    ins = []
    for c in range(NCORES):
        xc = np.zeros((D_IN, NODEPAD), BF)
        xc[:, :2500] = xt[:, c * 2500:(c + 1) * 2500]
        ins.append(dict(xT=xc, w_in=w_in, w_gat=w_gat, w_sr=w_sr,
                        v_sd=v_sd, b_in=b_in))
    return ins


def prepare_b_inputs(fp, g, h, hg, hsr, a_sd, masks_all):
    a_src = np.ascontiguousarray(a_sd[:, :4])
    a_dst = np.ascontiguousarray(a_sd[:, 4:])
    table = np.zeros((N, TBL_W), BF)
    table[:, :D_H] = h.astype(BF)
    table[:, D_H:2 * D_H] = hg
    table[:, EXTRA:EXTRA + 8] = a_src.astype('<f4').view(BF)

    bias = np.zeros((128, 32), np.float32)

    def putb(vec, col):
        v = np.asarray(vec, np.float32).reshape(-1, 128).T
        bias[:, col:col + v.shape[1]] = v
    putb(fp['b_gcn'], 0)
    putb(fp['b_sage'], 4)
    putb(fp['res'][0]['b1'], 8)
    putb(fp['res'][0]['b2'], 12)
    putb(fp['res'][1]['b1'], 16)
    putb(fp['res'][1]['b2'], 20)
    putb(fp['bc1'], 24)
    bias[0, 26] = fp['bc2'][0]

    wcommon = dict(
        w_gcn=fp['w_gcn'].astype(BF), w_sl=fp['w_sage_l'].astype(BF),
        r1w1=fp['res'][0]['w1'].astype(BF), r1w2=fp['res'][0]['w2'].astype(BF),
        r2w1=fp['res'][1]['w1'].astype(BF), r2w2=fp['res'][1]['w2'].astype(BF),
        wc1=np.ascontiguousarray(fp['wc1'], dtype=np.float32),
        wc2=np.ascontiguousarray(fp['wc2'], dtype=np.float32),
        bias=bias,
        bgat=np.ascontiguousarray(
            np.tile(fp['b_gat'][None, :], (128, 1)).astype(np.float32)),
        tbl=table)

    dinv, rcnt = g['dinv'], g['rcnt']
    ins = []
    slot = np.arange(NSLOT)
    b_ = slot // BLK_DST
    j_ = slot % BLK_DST
    p_ = b_ // 2
    q_ = b_ % 2
    Ln = q_ * 64 + j_
    Lf = (1 - q_) * 64 + j_
    for c in range(NCORES):
        s2n = g['slot2node'][c]
        valid = s2n >= 0
        s2nc = np.where(valid, s2n, 0)
        gloc_ = g['gloc'][c]
        gdst_ = g['gdst'][c]
        pad = gloc_ < 0

        gidx_img = idx_image(g['gsrc'][c])
        adst_e = np.where(pad[:, None], np.float32(PAD_ADST),
                          a_dst[np.where(pad, 0, gdst_)]).astype(np.float32)
        adst_img = np.ascontiguousarray(
            adst_e.reshape(T, 128, 4).transpose(1, 0, 2))

        aown = np.zeros((128, NPAIR, 8), np.float32)
        aown[Ln, p_, :] = np.where(valid[:, None],
                                   np.concatenate([a_src, a_dst], 1)[s2nc], 0)
        hgown = np.zeros((128, NPAIR, D_H), BF)
        hgown[Ln, p_, :] = np.where(valid[:, None], hg[s2nc].astype(np.float32),
                                    0).astype(BF)
        hown2 = np.zeros((128, NPAIR, D_H), BF)
        hown2[Lf, p_, :] = np.where(valid[:, None], h[s2nc], 0).astype(BF)
        scal = np.zeros((128, NPAIR, 4), np.float32)
        scal[Ln, p_, 0] = np.where(valid, rcnt[s2nc], 0)
        scal[Lf, p_, 1] = np.where(valid, dinv[s2nc], 0)
        hsr_slot = np.zeros((D_H, NSLOT), BF)
        hsr_slot[:, slot[valid]] = hsr[s2n[valid]].T

        d = dict(wcommon)
        d.update(gidx=gidx_img, masks=np.ascontiguousarray(masks_all[c]),
                 adst=adst_img, aown=aown, hgown=hgown, hown2=hown2,
                 scal=scal, hsrT=np.ascontiguousarray(hsr_slot))
        ins.append(d)
    return ins


def kernel(x, edge_index, params, debug=False, return_aux=False,
           timeit=0):
    x = np.asarray(x)
    edge_index = np.asarray(edge_index)
    nc_a, nc_b = _get_programs(debug=debug)
    fp = fold_params(params)
    g = build_graph_layout(edge_index)
    masks_all = build_masks(g)

    run_a = _get_runner("a", debug=debug)
    run_b = _get_runner("b", debug=debug)
    ins_a = prepare_a_inputs(x, fp)
    res_a_list, t_a = run_a(ins_a, timeit=timeit)
    h = np.zeros((N, D_H), np.float32)
    hg = np.zeros((N, D_H), BF)
    hsr = np.zeros((N, D_H), BF)
    a_sd = np.zeros((N, 8), np.float32)
    for c in range(NCORES):
        r = res_a_list[c]
        sl = slice(c * 2500, (c + 1) * 2500)
        h[sl] = np.asarray(r['hT'], np.float32).T[:2500]
        hg[sl] = np.asarray(r['hgT']).T[:2500].astype(BF)
        hsr[sl] = np.asarray(r['hsrT']).T[:2500].astype(BF)
        a_sd[sl] = np.asarray(r['aT'], np.float32).T[:2500]

    ins_b = prepare_b_inputs(fp, g, h, hg, hsr, a_sd, masks_all)
    res_b_list, t_b = run_b(ins_b, timeit=timeit)
    if timeit:
        _CACHE["last_times"] = (t_a, t_b)
        print(f"launch A avg: {t_a*1e9:.0f} ns  launch B avg: {t_b*1e9:.0f} ns")

    out = np.zeros(N, np.float32)
    for c in range(NCORES):
        s2n = g['slot2node'][c]
        valid = s2n >= 0
        lg = np.asarray(res_b_list[c]['logits'], np.float32).reshape(-1)
        out[s2n[valid]] = lg[valid]
    if return_aux:
        class _R:
            pass
        ra = _R(); ra.results = res_a_list
        rb = _R(); rb.results = res_b_list
        return out, dict(res_a=ra, res_b=rb, g=g, fp=fp, h=h, hg=hg,
                         hsr=hsr, a_sd=a_sd)
    return out
